# revision 1
# baseline (speedup 1.0000x reference)
"""Trainium2 Bass kernel for AutoRegressiveAdaptiveSpectralConv2d.

reference:  f = fft2(x)[..., :32, :32]
            o = einsum('btixy,tioxy->btoxy', f, R_w) * Ws_w
            o = (o * Wt/sum(Wt)).sum(t)            -> [B,1,U,32,32]
            out = ifft2(o, s=(256,256))            -> [B,1,U,256,256] complex64

Device decomposition (8 cores, single SPMD launch, fp16 data / fp32 PSUM):
  phase 1 (FFT, sharded over the 24 (b,t) pairs, 3 per core):
      PQT[w,m] = sum_h x[h,w]*ATs[h,m]           (ATs cols = [cos | -sin])
      f[re|im] = P@[cos|-sin] + Q@[sin|cos]      (4 matmuls accumulated in PSUM)
  AllToAll #1: redistribute f from (b,t)-sharding to xy-sharding
  phase 2 (channel mix, sharded over the 1024 (kx,ky) frequencies, 128 per core):
      per xy: O[m2,b] = sum_k W2[xy][k,m2]*fvec[k,b]  (k=(t,i,re/im), m2=(re/im,o))
  AllToAll #2: redistribute O from xy-sharding to (b,o)-sharding
  phase 3 (zero-padded iFFT, 16 (b,o) channels per core):
      G[re|im] = Or@[cos|sin] + Oi@[-sin|cos]    (PSUM-fused)
      out[re|im planes] = C-combos @ G           (PSUM-fused, fp16 planes out)
Weights pre-scaled by 2^22 on host (R values ~1e-7 would be fp16-subnormal);
iFFT matrices carry no 1/HW; host divides output by 2^38.
"""
import sys
import numpy as np

sys.path.insert(0, "/opt/trn_rl_repo")

import concourse.bass as bass
import concourse.bacc as bacc
import concourse.mybir as mybir
import concourse.tile as tile
from concourse import bass_utils

B, T, U, H, W = 4, 6, 32, 256, 256
MX, MY = 32, 32
NC = 8
PAIRS_PER_CORE = (B * T) // NC          # 3
CH_PER_CORE = (B * U) // NC             # 16
XY_PER_CORE = (MX * MY) // NC           # 128
K = 2 * T * U                           # 384 contraction rows (t,i,comp)
W_SCALE = float(2 ** 22)
OUT_DESCALE = float(2 ** 22) * float(H * W)

F16 = mybir.dt.float16
F32 = mybir.dt.float32

BODY_REPS = 1  # >1: repeat the whole body (for differential HW timing)


def _ap(t, offset, dims):
    """AP on a pool tile with explicit [step, count] dims (tile-relative)."""
    return bass.AP(t[:].tensor, offset, dims)


def build_nc(timing=False, local_exchange=False):
    nc = bacc.Bacc("TRN2", target_bir_lowering=False, debug=False, num_devices=NC)

    def ext_in(name, shape, dt):
        if timing:
            return nc.dram_tensor(name, shape, dt)
        return nc.dram_tensor(name, shape, dt, kind="ExternalInput")

    xsh = ext_in("xsh", [PAIRS_PER_CORE, U, H, W], F16)
    W2d = ext_in("W2d", [4, 3, 128, 2048], F16)
    ATs = ext_in("ATs", [H, 64], F16)
    ATq = ext_in("ATq", [H, 64], F16)
    CsT = ext_in("CsT", [MX, 512], F16)
    CsQ = ext_in("CsQ", [MX, 512], F16)
    CTm = ext_in("CTm", [MX, 768], F16)
    if timing:
        outp = nc.dram_tensor("outp", [CH_PER_CORE, 2, H, W], F16)
        dummy_in = nc.dram_tensor("dummy_in", [1, 64], F16, kind="ExternalInput")
        dummy_out = nc.dram_tensor("dummy_out", [1, 64], F16, kind="ExternalOutput")
    else:
        outp = nc.dram_tensor("outp", [CH_PER_CORE, 2, H, W], F16,
                              kind="ExternalOutput")

    with tile.TileContext(nc) as tc:
        with (
            tc.tile_pool(name="dram", bufs=1, space="DRAM") as dram,
            tc.tile_pool(name="consts", bufs=1) as consts,
            tc.tile_pool(name="p1sb", bufs=4) as p1sb,
            tc.tile_pool(name="p1st", bufs=2) as p1st,
            tc.tile_pool(name="p2fk", bufs=1) as p2fk,
            tc.tile_pool(name="p2w", bufs=4) as p2w,
            tc.tile_pool(name="p3sb", bufs=4) as p3sb,
        ):
            f_in = dram.tile([NC * 192, XY_PER_CORE], F16)
            f_out = dram.tile([NC * 192, XY_PER_CORE], F16)
            o_in = dram.tile([NC * 32, 128], F16)
            o_out = dram.tile([NC * 32, 128], F16)

            ats = consts.tile([128, 128], F16)      # [p, (hc, m)] h = hc*128+p
            nc.sync.dma_start(
                _ap(ats, 0, [[128, 128], [64, 2], [1, 64]]),
                bass.AP(ATs, 0, [[64, 128], [8192, 2], [1, 64]]),
            )
            atq = consts.tile([128, 128], F16)
            nc.sync.dma_start(
                _ap(atq, 0, [[128, 128], [64, 2], [1, 64]]),
                bass.AP(ATq, 0, [[64, 128], [8192, 2], [1, 64]]),
            )
            cst = consts.tile([MX, 512], F16)
            nc.scalar.dma_start(cst[:], CsT[:])
            csq = consts.tile([MX, 512], F16)
            nc.scalar.dma_start(csq[:], CsQ[:])
            ctm = consts.tile([MX, 768], F16)
            nc.scalar.dma_start(ctm[:], CTm[:])

            for _rep in range(BODY_REPS):
                # ================= phase 1: truncated FFT =================
                p1ctx_a = tc.tile_pool(name="p1ps_a", bufs=4, space="PSUM")
                p1ps_a = p1ctx_a.__enter__()
                p1ctx_b = tc.tile_pool(name="p1ps_b", bufs=4, space="PSUM")
                p1ps_b = p1ctx_b.__enter__()
                eng = 0
                for btl in range(PAIRS_PER_CORE):
                    f_stage = p1st.tile([MX, U * 64], F16, tag="fstage")
                    for i in range(U):
                        if i % 2 == 0:
                            x_sb = p1sb.tile([128, 1024], F16, tag="x")
                            nc.sync.dma_start(
                                _ap(x_sb, 0, [[1024, 128], [256, 4], [1, 256]]),
                                bass.AP(xsh, (btl * U + i) * H * W,
                                        [[256, 128], [128 * 256, 4], [1, 256]]),
                            )
                        xoff = (i % 2) * 512
                        pqt_sb = p1sb.tile([128, 128], F16, tag="pqt")
                        pqt_ps = p1ps_a.tile([128, 128], F32, tag="pqt_ps")
                        for wc in range(2):
                            for hc in range(2):
                                nc.tensor.matmul(
                                    _ap(pqt_ps, wc * 64, [[128, 128], [1, 64]]),
                                    _ap(x_sb, xoff + hc * 256 + wc * 128,
                                        [[1024, 128], [1, 128]]),
                                    _ap(ats, hc * 64, [[128, 128], [1, 64]]),
                                    start=(hc == 0), stop=(hc == 1),
                                )
                        cp = nc.scalar.copy if (eng % 2 == 0) else nc.vector.tensor_copy
                        cp(pqt_sb[:], pqt_ps[:])
                        # f[re|im] = P@[cos|-sin] + Q@[sin|cos], PSUM-accumulated
                        f_ps = p1ps_b.tile([MX, 64], F32, tag="f_ps")
                        step = 0
                        for half in range(2):   # 0: P-cols, 1: Q-cols
                            rhs_c = ats if half == 0 else atq
                            for wc in range(2):
                                nc.tensor.matmul(
                                    f_ps[:],
                                    _ap(pqt_sb, wc * 64 + half * 32,
                                        [[128, 128], [1, 32]]),
                                    _ap(rhs_c, wc * 64, [[128, 128], [1, 64]]),
                                    start=(step == 0), stop=(step == 3),
                                )
                                step += 1
                        cp = nc.scalar.copy if (eng % 2 == 1) else nc.vector.tensor_copy
                        cp(_ap(f_stage, i * 64, [[U * 64, MX], [1, 64]]), f_ps[:])
                        eng += 1
                    for dst in range(NC):
                        eng_dma = nc.sync if dst % 2 == 0 else nc.scalar
                        eng_dma.dma_start(
                            bass.AP(f_in[:].tensor, dst * 24576 + btl * 8192,
                                    [[32, 4], [128, 64], [1, 32]]),
                            _ap(f_stage, dst * 4 * 2048, [[2048, 4], [1, 2048]]),
                        )
                p1ctx_b.__exit__(None, None, None)
                p1ctx_a.__exit__(None, None, None)

                if local_exchange:
                    nc.sync.dma_start(f_out[:], f_in[:])
                else:
                    nc.gpsimd.collective_compute(
                        "AllToAll", mybir.AluOpType.bypass,
                        replica_groups=[list(range(NC))],
                        ins=[f_in.opt()], outs=[f_out.opt()],
                    )

                # ================= phase 2: spectral channel mixing =============
                p2ctx = tc.tile_pool(name="p2ps", bufs=2, space="PSUM")
                p2ps = p2ctx.__enter__()
                fk = p2fk.tile([128, 3 * B * XY_PER_CORE], F16, tag="fk")
                for b in range(B):
                    nc.sync.dma_start(
                        _ap(fk, b * 128, [[1536, 128], [512, 3], [1, 128]]),
                        bass.AP(f_out[:].tensor, b * 384 * 128,
                                [[128, 128], [128 * 128, 3], [1, 128]]),
                    )
                o_sb = p2fk.tile([64, 512], F16, tag="osb")  # [(set,o),(b,ky,kxl)]
                for kxl in range(4):
                    w_sb = p2w.tile([128, 32 * 192], F16, tag="wsb")
                    nc.scalar.dma_start(
                        _ap(w_sb, 0, [[6144, 128], [2048, 3], [1, 2048]]),
                        bass.AP(W2d, kxl * 3 * 128 * 2048,
                                [[2048, 128], [128 * 2048, 3], [1, 2048]]),
                    )
                    o_ps = p2ps.tile([64, 128], F32, tag="o_ps")
                    for kyy in range(32):
                        for kc in range(3):
                            nc.tensor.matmul(
                                _ap(o_ps, kyy * 4, [[128, 64], [1, 4]]),
                                _ap(w_sb, kc * 2048 + kyy * 64,
                                    [[6144, 128], [1, 64]]),
                                _ap(fk, kc * 512 + kxl * 32 + kyy,
                                    [[1536, 128], [128, 4]]),
                                start=(kc == 0), stop=(kc == 2),
                            )
                    cp = nc.scalar.copy if (kxl % 2 == 0) else nc.vector.tensor_copy
                    cp(
                        _ap(o_sb, kxl, [[512, 64], [4, 32], [128, 4]]),
                        _ap(o_ps, 0, [[128, 64], [4, 32], [1, 4]]),
                    )
                for b in range(B):
                    for st in range(2):
                        for ohi in range(2):
                            eng_dma = nc.sync if (b + st) % 2 == 0 else nc.scalar
                            eng_dma.dma_start(
                                bass.AP(o_in[:].tensor,
                                        (2 * b + ohi) * 4096 + st * 4,
                                        [[8, 16], [128, 32], [1, 4]]),
                                _ap(o_sb, (st * 32 + ohi * 16) * 512 + b * 128,
                                    [[512, 16], [4, 32], [1, 4]]),
                            )
                p2ctx.__exit__(None, None, None)

                if local_exchange:
                    nc.sync.dma_start(o_out[:], o_in[:])
                else:
                    nc.gpsimd.collective_compute(
                        "AllToAll", mybir.AluOpType.bypass,
                        replica_groups=[list(range(NC))],
                        ins=[o_in.opt()], outs=[o_out.opt()],
                    )

                # ================= phase 3: zero-padded iFFT ====================
                p3ctx_a = tc.tile_pool(name="p3ps_a", bufs=3, space="PSUM")
                p3ps_a = p3ctx_a.__enter__()
                p3ctx_b = tc.tile_pool(name="p3ps_b", bufs=3, space="PSUM")
                p3ps_b = p3ctx_b.__enter__()
                ot = consts.tile([MX, CH_PER_CORE * 64], F16)
                for comp in range(2):
                    for sr in range(NC):
                        eng_dma = nc.sync if sr % 2 == 0 else nc.scalar
                        eng_dma.dma_start(
                            _ap(ot, comp * 32 + sr * 4,
                                [[1024, 32], [64, 16], [1, 4]]),
                            bass.AP(o_out[:].tensor, sr * 4096 + comp * 4,
                                    [[128, 32], [8, 16], [1, 4]]),
                        )
                for chl in range(CH_PER_CORE):
                    # G[re|im] = Or@[cos|sin] + Oi@[-sin|cos], PSUM-fused
                    g_ps = p3ps_b.tile([MX, 512], F32, tag="g_ps")
                    nc.tensor.matmul(
                        g_ps[:],
                        _ap(ot, chl * 64, [[1024, 32], [4, 8], [1, 4]]),
                        cst[:], start=True, stop=False)
                    nc.tensor.matmul(
                        g_ps[:],
                        _ap(ot, chl * 64 + 32, [[1024, 32], [4, 8], [1, 4]]),
                        csq[:], start=False, stop=True)
                    g_sb = p3sb.tile([MX, 512], F16, tag="gsb")
                    cp = nc.scalar.copy if (chl % 2 == 0) else nc.vector.tensor_copy
                    cp(g_sb[:], g_ps[:])
                    for hc in range(2):
                        # p_ps[re cols | im cols], one PSUM group of 4 matmuls
                        p_ps = p3ps_a.tile([128, 512], F32, tag="p_ps")
                        nc.tensor.matmul(
                            _ap(p_ps, 0, [[512, 128], [1, 256]]),
                            _ap(ctm, 0 * 256 + hc * 128, [[768, MX], [1, 128]]),
                            _ap(g_sb, 0, [[512, MX], [1, 256]]),
                            start=True, stop=False)
                        nc.tensor.matmul(
                            _ap(p_ps, 0, [[512, 128], [1, 256]]),
                            _ap(ctm, 2 * 256 + hc * 128, [[768, MX], [1, 128]]),
                            _ap(g_sb, 256, [[512, MX], [1, 256]]),
                            start=False, stop=True)
                        nc.tensor.matmul(
                            _ap(p_ps, 256, [[512, 128], [1, 256]]),
                            _ap(ctm, 1 * 256 + hc * 128, [[768, MX], [1, 128]]),
                            _ap(g_sb, 0, [[512, MX], [1, 256]]),
                            start=True, stop=False)
                        nc.tensor.matmul(
                            _ap(p_ps, 256, [[512, 128], [1, 256]]),
                            _ap(ctm, 0 * 256 + hc * 128, [[768, MX], [1, 128]]),
                            _ap(g_sb, 256, [[512, MX], [1, 256]]),
                            start=False, stop=True)
                        out_int = p3sb.tile([128, 512], F16, tag="oint")
                        cp = nc.scalar.copy if (hc == 0) else nc.vector.tensor_copy
                        cp(out_int[:], p_ps[:])
                        nc.sync.dma_start(
                            bass.AP(outp, chl * 2 * H * W + hc * 128 * W,
                                    [[256, 128], [65536, 2], [1, 256]]),
                            _ap(out_int, 0, [[512, 128], [256, 2], [1, 256]]),
                        )
                p3ctx_b.__exit__(None, None, None)
                p3ctx_a.__exit__(None, None, None)
            if timing:
                nc.sync.dma_start(bass.AP(dummy_out, 0, [[64, 1], [1, 64]]),
                                  bass.AP(dummy_in, 0, [[64, 1], [1, 64]]))
    nc.compile()
    return nc


_NC_CACHE = None


def _get_nc():
    global _NC_CACHE
    if _NC_CACHE is None:
        _NC_CACHE = build_nc()
    return _NC_CACHE


def _host_prep(x, R_w, Ws_w, Wt_w):
    x = np.asarray(x)
    R_w = np.asarray(R_w)
    Ws_w = np.asarray(Ws_w, dtype=np.float32)
    Wt_w = np.asarray(Wt_w, dtype=np.float32)

    xf = x.reshape(B * T, U, H, W).astype(np.float16)

    h = np.arange(H)[:, None]
    k = np.arange(MX)[None, :]
    ang = 2.0 * np.pi * h * k / H
    ATs = np.concatenate([np.cos(ang), -np.sin(ang)], axis=1).astype(np.float16)
    ATq = np.concatenate([np.sin(ang), np.cos(ang)], axis=1).astype(np.float16)

    xg = np.arange(MX)[:, None]
    wg = np.arange(W)[None, :]
    ang2 = 2.0 * np.pi * xg * wg / W
    cos2 = np.cos(ang2).astype(np.float32)
    sin2 = np.sin(ang2).astype(np.float32)
    CsT = np.concatenate([cos2, sin2], axis=1).astype(np.float16)
    CsQ = np.concatenate([-sin2, cos2], axis=1).astype(np.float16)
    CTm = np.concatenate([cos2, sin2, -sin2], axis=1).astype(np.float16)

    wt = (Wt_w / Wt_w.sum()).reshape(T)
    Wc = (R_w * Ws_w[None, None, None]
          * wt[:, None, None, None, None].astype(np.float32) * W_SCALE)
    Wr = np.ascontiguousarray(np.real(Wc), dtype=np.float32)
    Wi = np.ascontiguousarray(np.imag(Wc), dtype=np.float32)
    Wr_f = Wr.transpose(3, 4, 0, 1, 2).reshape(MX * MY, T * U, U)
    Wi_f = Wi.transpose(3, 4, 0, 1, 2).reshape(MX * MY, T * U, U)
    W2 = np.empty((MX * MY, K, 64), np.float16)
    W2[:, 0::2, 0:U] = Wr_f
    W2[:, 1::2, 0:U] = -Wi_f
    W2[:, 0::2, U:] = Wi_f
    W2[:, 1::2, U:] = Wr_f

    in_maps = []
    for c in range(NC):
        in_maps.append({
            "xsh": np.ascontiguousarray(xf[c * PAIRS_PER_CORE:(c + 1) * PAIRS_PER_CORE]),
            "W2d": np.ascontiguousarray(
                W2[c * XY_PER_CORE:(c + 1) * XY_PER_CORE]
                .reshape(4, 32, 3, 128, 64).transpose(0, 2, 3, 1, 4)
                .reshape(4, 3, 128, 2048)),
            "ATs": ATs,
            "ATq": ATq,
            "CsT": CsT,
            "CsQ": CsQ,
            "CTm": CTm,
        })
    return in_maps


def _host_post(results):
    out = np.empty((B, 1, U, H, W), np.complex64)
    inv = np.float32(1.0 / OUT_DESCALE)
    for c in range(NC):
        arr = np.asarray(results[c]["outp"]).astype(np.float32)  # [16,2,256,256]
        carr = (arr[:, 0] + 1j * arr[:, 1]).astype(np.complex64)
        for j in range(CH_PER_CORE):
            ch = c * CH_PER_CORE + j
            out[ch // U, 0, ch % U] = carr[j] * inv
    return out


def kernel(**inputs):
    nc = _get_nc()
    in_maps = _host_prep(inputs["input"], inputs["R_w"], inputs["Ws_w"], inputs["Wt_w"])
    res = bass_utils.run_bass_kernel_spmd(nc, in_maps, core_ids=list(range(NC)))
    return _host_post(res.results)

